# revision 2
# baseline (speedup 1.0000x reference)
"""Trainium2 Bass kernel for nn_LocalDynamics (GNN message passing).

Design: shard destination nodes across 8 cores (12500 each). Each message
(LF: dest=line_from, LT: dest=line_to, GB: dest=gen_bus) is routed to the
core owning its destination, sorted by local dest, and padded so no dest
segment crosses a 128-token block. On device, per 1024-token tile:
  - dma_gather the dest rows (local int16 idx) and indirect-DMA the other
    endpoint rows (global int32 idx) into one [128,128] f32 tile per block,
  - PE-transpose to feature-major, run the 3-layer MLP in bf16,
  - dedup duplicate dests per block with an is_equal selection-matrix matmul
    (each token gets its dest-segment's sum), then dma_scatter_add only the
    segment-head tokens into the core's delta slice (heads are unique per
    instruction, so the CCE add never races); non-heads go to a dummy row.
Final pass applies tanh(delta) -> out. Host concatenates the 8 slices.
"""
import os
import sys
import numpy as np

sys.path.insert(0, "/opt/trn_rl_repo")

from concourse import bass, bacc, mybir, tile
from concourse.bass_utils import run_bass_kernel_spmd
from concourse.masks import make_identity
import ml_dtypes

bf16 = ml_dtypes.bfloat16

N = 100000
NCORES = 8
SL = N // NCORES          # 12500 nodes per core
DN = 12544                # delta rows (= 98*128), dummy row below
DUMMY = 12543
T = 1024                  # tokens per tile
D = 64


def _wrap16(idx, n):
    a = np.asarray(idx, np.int16).reshape(n // 16, 16).T
    return np.tile(a, (8, 1)).copy()


def _pack_stream(dest_loc, other, xrows, S, xd, has_other):
    """dest-sort, block-align segments, pad to S. Returns per-core arrays."""
    order = np.argsort(dest_loc, kind="stable")
    d = dest_loc[order]
    oth = other[order] if has_other else None
    xr = xrows[order]
    n = len(d)
    if n:
        starts_mask = np.r_[True, d[1:] != d[:-1]]
        seg_starts = np.where(starts_mask)[0]
        seg_lens = np.diff(np.r_[seg_starts, n])
        assert seg_lens.max(initial=1) <= 128, "node degree > 128 unsupported"
        out_starts = np.empty(len(seg_starts), np.int64)
        pos = 0
        for i, L in enumerate(seg_lens):
            if (pos % 128) + L > 128:
                pos = (pos // 128 + 1) * 128
            out_starts[i] = pos
            pos += L
        positions = np.repeat(out_starts, seg_lens) + (
            np.arange(n) - np.repeat(seg_starts, seg_lens))
    else:
        out_starts = np.empty(0, np.int64)
        seg_starts = np.empty(0, np.int64)
        positions = np.empty(0, np.int64)
        pos = 0
    assert pos <= S, (pos, S)

    gidx = np.zeros(S, np.int32)
    gidx[positions] = d
    sidx = np.full(S, DUMMY, np.int32)
    if n:
        sidx[out_starts] = d[seg_starts]
    dval = (13000 + (np.arange(S) % 128)).astype(np.float32)
    dval[positions] = d.astype(np.float32)
    oth32 = np.zeros(S, np.int32)
    if has_other:
        oth32[positions] = oth
    x = np.zeros((S, xd), np.float32)
    x[positions] = xr

    out = {
        "gidx": _wrap16(gidx, S),
        "sidx": _wrap16(sidx, S),
        "dcol": dval.reshape(S // 128, 128).T.copy(),          # [128, S/128] f32
        "drow": dval.reshape(1, S),                             # [1, S] f32
        "x": np.ascontiguousarray(x.T).astype(bf16),            # [xd, S] bf16
    }
    if has_other:
        out["oidx"] = oth32.reshape(S // 128, 128).T.astype(np.int32).copy()
    return out


def _roundup(v, m):
    return (v + m - 1) // m * m


def _prep_weights(inputs):
    """Per-stream weight slices. act rows: [dest(64) | other(64)] then x."""
    ws = {}
    for s, pre, kd in (("lf", "lf", None), ("lt", "lt", None), ("gb", "gb", None)):
        w1 = np.asarray(inputs[f"{pre}_w1"], np.float32)
        if s == "lf":   # dest=from -> rows [0:64]; other=to -> rows [64:128]
            w1do = np.vstack([w1[0:64], w1[64:128]])
            w1x, w1gt = w1[128:136], w1[136:153]
        elif s == "lt":  # dest=to -> rows [64:128]; other=from -> rows [0:64]
            w1do = np.vstack([w1[64:128], w1[0:64]])
            w1x, w1gt = w1[128:136], w1[136:153]
        else:            # gb: dest=bus rows [0:64]
            w1do = w1[0:64]
            w1x, w1gt = w1[64:68], w1[68:85]
        ws[s] = {
            "w1do": w1do.astype(bf16), "w1x": w1x.astype(bf16),
            "w1gt": w1gt.astype(np.float32),
            "b1": np.asarray(inputs[f"{pre}_b1"], np.float32).reshape(128, 1),
            "w2": np.asarray(inputs[f"{pre}_w2"], np.float32).astype(bf16),
            "b2": np.asarray(inputs[f"{pre}_b2"], np.float32).reshape(128, 1),
            "w3": np.asarray(inputs[f"{pre}_w3"], np.float32).astype(bf16),
            "b3": np.asarray(inputs[f"{pre}_b3"], np.float32).reshape(1, 64),
        }
    return ws


def _build(nc, sizes, xdims):
    """Construct the SPMD program. sizes: {s: S_s}."""
    f32, i16, i32, bfd = (mybir.dt.float32, mybir.dt.int16, mybir.dt.int32,
                          mybir.dt.bfloat16)
    t_hfull = nc.dram_tensor("h_full", [N, D], f32, kind="ExternalInput")
    t_hslice = nc.dram_tensor("h_slice", [SL, D], f32, kind="ExternalInput")
    t_hgt = nc.dram_tensor("hgt", [17, 1], f32, kind="ExternalInput")
    t_delta = nc.dram_tensor("delta", [DN, D], f32, kind="ExternalOutput")
    t_out = nc.dram_tensor("out", [DN, D], f32, kind="ExternalOutput")

    P = {}
    for s in ("lf", "lt", "gb"):
        S, xd = sizes[s], xdims[s]
        kh = 128 if s != "gb" else 64
        P[s] = {
            "gidx": nc.dram_tensor(f"{s}_gidx", [128, S // 16], i16, kind="ExternalInput"),
            "sidx": nc.dram_tensor(f"{s}_sidx", [128, S // 16], i16, kind="ExternalInput"),
            "dcol": nc.dram_tensor(f"{s}_dcol", [128, S // 128], f32, kind="ExternalInput"),
            "drow": nc.dram_tensor(f"{s}_drow", [1, S], f32, kind="ExternalInput"),
            "x": nc.dram_tensor(f"{s}_x", [xd, S], bfd, kind="ExternalInput"),
            "w1do": nc.dram_tensor(f"{s}_w1do", [kh, 128], bfd, kind="ExternalInput"),
            "w1x": nc.dram_tensor(f"{s}_w1x", [xd, 128], bfd, kind="ExternalInput"),
            "w1gt": nc.dram_tensor(f"{s}_w1gt", [17, 128], f32, kind="ExternalInput"),
            "b1": nc.dram_tensor(f"{s}_b1", [128, 1], f32, kind="ExternalInput"),
            "w2": nc.dram_tensor(f"{s}_w2", [128, 128], bfd, kind="ExternalInput"),
            "b2": nc.dram_tensor(f"{s}_b2", [128, 1], f32, kind="ExternalInput"),
            "w3": nc.dram_tensor(f"{s}_w3", [128, 64], bfd, kind="ExternalInput"),
            "b3": nc.dram_tensor(f"{s}_b3", [1, 64], f32, kind="ExternalInput"),
        }
        if s != "gb":
            P[s]["oidx"] = nc.dram_tensor(f"{s}_oidx", [128, S // 128], i32, kind="ExternalInput")

    with tile.TileContext(nc) as tc:
        with (
            tc.tile_pool(name="const", bufs=1) as cpool,
            tc.tile_pool(name="idx", bufs=1) as ipool,
            tc.tile_pool(name="work", bufs=3) as wpool,
            tc.tile_pool(name="psum", bufs=2, space="PSUM") as ppool,
        ):
            ident = cpool.tile([128, 128], f32)
            make_identity(nc, ident[:])
            ones1 = cpool.tile([1, 128], f32)
            nc.gpsimd.memset(ones1[:], 1.0)
            hgt_sb = cpool.tile([17, 1], f32)
            nc.sync.dma_start(hgt_sb[:], t_hgt[:])

            for s in ("lf", "lt", "gb"):
                S, xd = sizes[s], xdims[s]
                kh = 128 if s != "gb" else 64
                has_oth = s != "gb"
                pp = P[s]

                w1do = cpool.tile([kh, 128], bfd, tag=f"{s}w1do")
                w1x = cpool.tile([xd, 128], bfd, tag=f"{s}w1x")
                w1gt = cpool.tile([17, 128], f32, tag=f"{s}w1gt")
                b1t = cpool.tile([128, 1], f32, tag=f"{s}b1")
                w2 = cpool.tile([128, 128], bfd, tag=f"{s}w2")
                b2t = cpool.tile([128, 1], f32, tag=f"{s}b2")
                w3 = cpool.tile([128, 64], bfd, tag=f"{s}w3")
                b3t = cpool.tile([1, 64], f32, tag=f"{s}b3")
                for tl, pr in ((w1do, "w1do"), (w1x, "w1x"), (w1gt, "w1gt"),
                               (b1t, "b1"), (w2, "w2"), (b2t, "b2"),
                               (w3, "w3"), (b3t, "b3")):
                    nc.sync.dma_start(tl[:], pp[pr][:])

                # bias1 = b1 + [hg;t] @ w1gt ; b3 replicated to 128 partitions
                pb = ppool.tile([128, 512], f32, tag="p1")
                nc.tensor.matmul(pb[:, 0:1], w1gt[:], hgt_sb[:], start=True, stop=True)
                bias1 = cpool.tile([128, 1], f32, tag=f"{s}bias1")
                nc.vector.tensor_tensor(out=bias1[:], in0=pb[:, 0:1], in1=b1t[:],
                                        op=mybir.AluOpType.add)
                pb2 = ppool.tile([128, 512], f32, tag="p1")
                nc.tensor.matmul(pb2[:, 0:64], ones1[:], b3t[:], start=True, stop=True)
                b3rep = cpool.tile([128, 64], f32, tag=f"{s}b3rep")
                nc.vector.tensor_copy(b3rep[:], pb2[:, 0:64])

                gidx = ipool.tile([128, S // 16], i16, tag=f"{s}gidx")
                sidx = ipool.tile([128, S // 16], i16, tag=f"{s}sidx")
                dcol = ipool.tile([128, S // 128], f32, tag=f"{s}dcol")
                nc.sync.dma_start(gidx[:], pp["gidx"][:])
                nc.sync.dma_start(sidx[:], pp["sidx"][:])
                nc.sync.dma_start(dcol[:], pp["dcol"][:])
                if has_oth:
                    oidx = ipool.tile([128, S // 128], i32, tag=f"{s}oidx")
                    nc.sync.dma_start(oidx[:], pp["oidx"][:])

                ntile = S // T
                NB = T // 128
                for it in range(ntile):
                    c16, c128 = it * (T // 16), it * NB
                    gw = 128 if has_oth else 64
                    g = wpool.tile([128, NB, gw], f32, tag="g")
                    if has_oth:
                        gd = wpool.tile([128, NB, D], f32, tag="gd")
                        nc.gpsimd.dma_gather(
                            gd[:], t_hslice[:], gidx[:, c16:c16 + T // 16], T, T, D)
                        nc.vector.tensor_copy(g[:, :, 0:64], gd[:])
                    else:
                        nc.gpsimd.dma_gather(
                            g[:], t_hslice[:], gidx[:, c16:c16 + T // 16], T, T, D)
                    if has_oth:
                        for b in range(NB):
                            nc.gpsimd.indirect_dma_start(
                                out=g[:, b, 64:128], out_offset=None,
                                in_=t_hfull[:],
                                in_offset=bass.IndirectOffsetOnAxis(
                                    ap=oidx[:, c128 + b:c128 + b + 1], axis=0))
                    xa = wpool.tile([xd, T], bfd, tag="xa")
                    nc.sync.dma_start(xa[:], pp["x"][:, it * T:(it + 1) * T])

                    act = wpool.tile([gw, T], bfd, tag="act")
                    for b in range(NB):
                        pt = ppool.tile([128, 128], f32, tag="pt")
                        nc.tensor.transpose(pt[0:gw, :], g[:, b, :], ident[:])
                        nc.vector.tensor_copy(act[:, b * 128:(b + 1) * 128],
                                              pt[0:gw, :])

                    h1 = wpool.tile([128, T], bfd, tag="h1")
                    h2 = wpool.tile([128, T], bfd, tag="h2")
                    for half in range(T // 512):
                        hs = slice(half * 512, (half + 1) * 512)
                        p1 = ppool.tile([128, 512], f32, tag="p1")
                        nc.tensor.matmul(p1[:], w1do[:], act[:, hs], start=True, stop=False)
                        nc.tensor.matmul(p1[:], w1x[:], xa[:, hs], start=False, stop=True)
                        nc.scalar.activation(h1[:, hs], p1[:],
                                             mybir.ActivationFunctionType.Tanh,
                                             bias=bias1[:])
                        p2 = ppool.tile([128, 512], f32, tag="p1")
                        nc.tensor.matmul(p2[:], w2[:], h1[:, hs], start=True, stop=True)
                        nc.scalar.activation(h2[:, hs], p2[:],
                                             mybir.ActivationFunctionType.Tanh,
                                             bias=b2t[:])

                    p3 = ppool.tile([128, NB, 64], f32, tag="p3")
                    for b in range(NB):
                        nc.tensor.matmul(p3[:, b, :], h2[:, b * 128:(b + 1) * 128],
                                         w3[:], start=True, stop=True)
                    nc.vector.tensor_tensor(
                        out=p3[:], in0=p3[:],
                        in1=b3rep[:].unsqueeze(1).to_broadcast([128, NB, 64]),
                        op=mybir.AluOpType.add)
                    m = wpool.tile([128, NB, 64], bfd, tag="m")
                    nc.scalar.activation(m[:], p3[:],
                                         mybir.ActivationFunctionType.Tanh)

                    # selection matrix: sel[p, b, j] = (dest[b,p] == dest[b,j])
                    sel = wpool.tile([128, NB, 128], bfd, tag="sel")
                    rep = wpool.tile([128, T], f32, tag="rep")
                    dr = wpool.tile([1, T], f32, tag="dr")
                    nc.sync.dma_start(dr[:], pp["drow"][:, it * T:(it + 1) * T])
                    for half in range(T // 512):
                        hs = slice(half * 512, (half + 1) * 512)
                        pr = ppool.tile([128, 512], f32, tag="p1")
                        nc.tensor.matmul(pr[:], ones1[:], dr[:, hs],
                                         start=True, stop=True)
                        nc.vector.tensor_copy(rep[:, hs], pr[:])
                    nc.vector.tensor_tensor(
                        out=sel[:],
                        in0=dcol[:, c128:c128 + NB].unsqueeze(2).to_broadcast([128, NB, 128]),
                        in1=rep[:].rearrange("p (b j) -> p b j", b=NB),
                        op=mybir.AluOpType.is_equal)

                    pc = ppool.tile([128, NB, 64], f32, tag="pc")
                    for b in range(NB):
                        nc.tensor.matmul(pc[:, b, :], sel[:, b, :], m[:, b, :],
                                         start=True, stop=True)
                    outf = wpool.tile([128, NB, 64], f32, tag="outf")
                    nc.vector.tensor_copy(outf[:], pc[:])
                    nc.gpsimd.dma_scatter_add(
                        t_delta[:], outf[:], sidx[:, c16:c16 + T // 16], T, T, D)

            # final: out = tanh(delta)
            dview = t_delta.ap().rearrange("(b p) d -> p b d", p=128)
            oview = t_out.ap().rearrange("(b p) d -> p b d", p=128)
            for ch in range(14):
                dt_ = wpool.tile([128, 7, 64], f32, tag="fin")
                nc.sync.dma_start(dt_[:], dview[:, ch * 7:(ch + 1) * 7, :])
                ft = wpool.tile([128, 7, 64], f32, tag="fout")
                nc.scalar.activation(ft[:], dt_[:],
                                     mybir.ActivationFunctionType.Tanh)
                nc.sync.dma_start(oview[:, ch * 7:(ch + 1) * 7, :], ft[:])
    nc.compile()
    return nc


def kernel(**inputs):
    h_local = np.asarray(inputs["h_local"], np.float32)
    h_global = np.asarray(inputs["h_global"], np.float32).reshape(-1)
    x_line = np.asarray(inputs["x_line"], np.float32)
    x_gen = np.asarray(inputs["x_gen"], np.float32)
    tval = np.asarray(inputs["t"], np.float32).reshape(-1)
    line_from = np.asarray(inputs["line_from"], np.int64)
    line_to = np.asarray(inputs["line_to"], np.int64)
    gen_bus = np.asarray(inputs["gen_bus"], np.int64)

    streams = {
        "lf": (line_from, line_to, x_line, 8, True),
        "lt": (line_to, line_from, x_line, 8, True),
        "gb": (gen_bus, None, x_gen, 4, False),
    }
    # per-core packing; sizes = max padded length over cores, rounded to T
    packed = {s: [] for s in streams}
    sizes = {}
    xdims = {"lf": 8, "lt": 8, "gb": 4}
    for s, (dest, oth, xr, xd, has_oth) in streams.items():
        percore = []
        for c in range(NCORES):
            mask = (dest // SL) == c
            d_loc = (dest[mask] - c * SL).astype(np.int32)
            o = oth[mask].astype(np.int32) if has_oth else np.empty(0, np.int32)
            percore.append((d_loc, o, xr[mask]))
        # compute padded length per core (dry run of the greedy)
        maxlen = 0
        for d_loc, o, x in percore:
            order = np.argsort(d_loc, kind="stable")
            d = d_loc[order]
            if len(d):
                sm = np.r_[True, d[1:] != d[:-1]]
                lens = np.diff(np.r_[np.where(sm)[0], len(d)])
                pos = 0
                for L in lens:
                    if (pos % 128) + L > 128:
                        pos = (pos // 128 + 1) * 128
                    pos += L
            else:
                pos = 0
            maxlen = max(maxlen, pos)
        S = max(_roundup(maxlen, T), T)
        sizes[s] = S
        for d_loc, o, x in percore:
            packed[s].append(_pack_stream(d_loc, o, x, S, xd, has_oth))

    ws = _prep_weights(inputs)
    hgt = np.concatenate([h_global, tval]).reshape(17, 1).astype(np.float32)

    nc = bacc.Bacc("TRN2", target_bir_lowering=False, debug=False)
    nc = _build(nc, sizes, xdims)

    in_maps = []
    for c in range(NCORES):
        m = {"h_full": h_local,
             "h_slice": np.ascontiguousarray(h_local[c * SL:(c + 1) * SL]),
             "hgt": hgt}
        for s in streams:
            for k, v in packed[s][c].items():
                m[f"{s}_{k}"] = v
            for k, v in ws[s].items():
                m[f"{s}_{k}"] = v
        in_maps.append(m)

    trace = bool(os.environ.get("BASS_KERNEL_TRACE"))
    res = run_bass_kernel_spmd(nc, in_maps, core_ids=list(range(NCORES)),
                               trace=trace)
    if trace and res.exec_time_ns:
        print(f"HW exec time: {res.exec_time_ns} ns")
        if res.instructions_and_trace:
            print(f"trace path: {res.instructions_and_trace[1]}")
    out = np.concatenate([res.results[c]["out"][:SL] for c in range(NCORES)], 0)
    return out.astype(np.float32)

